# revision 48
# baseline (speedup 1.0000x reference)
"""Causal self-attention (B=2, T=4096, C=768, H=12) on 8 trn2 NeuronCores.

Sharding: core c handles batch b = c//4 and the 3 heads of head-group
hg = c%4 (tensor parallel over heads, data parallel over batch).  Each core
computes the qkv projection for its heads, causal attention, and a partial
output projection; the host sums the 4 per-head-group partials per batch.

v2 pipeline (granule conveyor):
  - Scores are computed transposed (S^T[tk, tq] = K Q^T, d=64 contraction).
    One PSUM "granule" = [128, 3, 512] (3 banks) holds the S^T block of all
    THREE heads for one 128-wide k-block: same causal trim for all slots, so
    ONE 1536-col ACT exp instruction covers the whole granule (the ~260ns
    per-instruction ACT overhead was 28% of scalar-engine time with per-side
    1024-col exps).
  - Granules double-buffer across two 3-bank homes (pool bufs=2) so the ACT
    exp conveyor never waits for PE refill.
  - S^T matmuls alternate PE row groups strictly (h0 at partitions 0-63,
    h1 at 64-127, h2 on its duplicated kT slab picking whichever half keeps
    the alternation going).  Adjacent matmuls on different row groups
    run concurrently in the 128x128 array (row tiling), ~2x the serialized
    rate for these 64-contraction matmuls.
  - PV runs as one accumulation chain per head per chunk on a single PSUM
    bank (h0 woven into its own chunk's granule stream; h1/h2 of chunk qc
    carried into chunk qc+1's stream, so ACT never starves at chunk
    boundaries).  Softmax denominator rides as a 65th ones-column of V;
    normalization broadcasts the reciprocal row.
  - qkv projection (chunk qc+2) and output projection (chunk qc-1) chains
    run as PE filler on the one remaining PSUM bank.
"""

import ml_dtypes
import numpy as np

import concourse.bass as bass
import concourse.mybir as mybir
import concourse.tile as tile
from concourse import bacc

B, T, C, H, HD = 2, 4096, 768, 12, 64
F32 = mybir.dt.float32
BF16 = mybir.dt.bfloat16
N_CORES = 8
AF = mybir.ActivationFunctionType


def build_nc(seq_len: int = T) -> bass.Bass:
    assert seq_len % 512 == 0
    TCH = seq_len // 512   # 512-wide t-chunks
    TB = seq_len // 128    # 128-wide t-blocks

    nc = bacc.Bacc(num_devices=N_CORES)

    xT = nc.dram_tensor("xT", (C, seq_len), BF16, kind="ExternalInput").ap()
    wqkT = nc.dram_tensor("wqkT", (C, 384), BF16, kind="ExternalInput").ap()
    wvT = nc.dram_tensor("wvT", (C, 192), BF16, kind="ExternalInput").ap()
    wpT = nc.dram_tensor("wpT", (192, C), BF16, kind="ExternalInput").ap()
    out = nc.dram_tensor("out", (seq_len, C), F32, kind="ExternalOutput").ap()

    with tile.TileContext(nc) as tc:
        with (
            tc.tile_pool(name="const", bufs=1) as const,
            tc.tile_pool(name="persist", bufs=1) as persist,
            tc.tile_pool(name="xt", bufs=2) as xtpool,
            tc.tile_pool(name="qt", bufs=3) as qtpool,
            tc.tile_pool(name="ot", bufs=3) as otpool,
            tc.tile_pool(name="p", bufs=34) as ppool,
            tc.tile_pool(name="small", bufs=4) as spool,
            tc.tile_pool(name="osb", bufs=3) as osbpool,
            tc.tile_pool(name="st", bufs=2, space="PSUM") as stpool,
            tc.tile_pool(name="qp", bufs=1, space="PSUM") as qppool,
            tc.tile_pool(name="pso", bufs=1, space="PSUM") as psopool,
        ):
            # ---- constants / weights ----
            # Startup-critical DMAs: per-cc block transfers (contiguous rows,
            # no gather) spread across FOUR engine queues so the ~600ns
            # per-descriptor issue cost parallelizes; the first qkv matmul
            # only waits on wqk[cc0] + xt0[cc0].
            wqk_sb = const.tile([128, 6, 384], BF16, tag="wqk")
            xt0 = xtpool.tile([128, 6, 512], BF16, tag="xt", name="xt")
            wv_sb = const.tile([128, 6, 192], BF16, tag="wv")
            for cc in range(6):
                nc.sync.dma_start(wqk_sb[:, cc, :], wqkT[cc * 128:(cc + 1) * 128, :])
                nc.gpsimd.dma_start(xt0[:, cc, :], xT[cc * 128:(cc + 1) * 128, 0:512])
                nc.scalar.dma_start(wv_sb[:, cc, :], wvT[cc * 128:(cc + 1) * 128, :])
            wp0_sb = const.tile([128, 768], BF16, tag="wp0")
            nc.scalar.dma_start(wp0_sb, wpT[0:128, :])
            # zero-pad wp1 to 128 partitions so the proj matmul stays K=128
            # (avoids a 64x128 <-> 128x128 PE mode switch per t-block)
            wp1_sb = const.tile([128, 768], BF16, tag="wp1")
            nc.vector.memset(wp1_sb[64:128, :], 0.0)
            nc.scalar.dma_start(wp1_sb[0:64, :], wpT[128:192, :])

            # emask[i, qw] = 1.0 if qw >= i else 0.0 (staircase for the one
            # 128x128 block straddling the causal diagonal)
            emask = const.tile([128, 128], BF16, tag="emask")
            nc.gpsimd.memset(emask, 1.0)
            nc.gpsimd.affine_select(
                out=emask, in_=emask,
                compare_op=mybir.AluOpType.is_ge,
                fill=0.0, base=0, pattern=[[1, 128]], channel_multiplier=-1,
            )

            # ---- persistent activations ----
            # kT slab0: h0 @ partitions 0-63, h1 @ 64-127.
            # slab1: h2 duplicated to both halves (lets h2's S^T matmul pick
            # either PE row group to keep the row-group alternation going).
            kT_sb = persist.tile([128, 2, seq_len], BF16, tag="kT")
            # v per head: [t-partition, kb, 64 dims + ones column]
            v_sb = [
                persist.tile([128, TB, 65], BF16, tag=f"v{h}", name=f"v{h}")
                for h in range(3)
            ]
            for h in range(3):
                nc.gpsimd.memset(v_sb[h][:, :, 64], 1.0)

            # per-chunk ring tiles (q / attention-out live one chunk only)
            qt_tiles: dict[int, object] = {}
            ot_tiles: dict[int, object] = {}
            # pt granule tiles: (qc, kb) -> (pt tile [128,3,512], qstart)
            pt_tiles: dict[tuple, tuple] = {}

            # ---- qkv projection chains (fillers) ----
            def make_qkv_fillers(tci, xt_pre=None):
                tcs = slice(tci * 512, (tci + 1) * 512)
                if xt_pre is not None:
                    xt = xt_pre
                else:
                    xt = xtpool.tile([128, 6, 512], BF16, tag="xt", name="xt")
                    for cc in range(6):
                        nc.sync.dma_start(
                            xt[:, cc, :], xT[cc * 128:(cc + 1) * 128, tcs]
                        )
                fs = []

                # q/k channels: m0=[q_h0|q_h1], m1=[k_h0|k_h1], m2=[q_h2|k_h2]
                def gm(m, ps=None, xt=xt, tci=tci, tcs=tcs):
                    if ps is None:
                        ps = qppool.tile([128, 512], F32, tag="qp", name="ps")
                    for cc in range(6):
                        nc.tensor.matmul(
                            ps,
                            lhsT=wqk_sb[:, cc, m * 128:(m + 1) * 128],
                            rhs=xt[:, cc, :],
                            start=(cc == 0), stop=(cc == 5),
                        )
                    if m == 0:
                        qt = qtpool.tile([128, 2, 512], BF16, tag="qt", name="qt")
                        qt_tiles[tci] = qt
                        nc.vector.tensor_copy(qt[:, 0, :], ps)
                    elif m == 1:
                        nc.vector.tensor_copy(kT_sb[:, 0, tcs], ps)
                    else:
                        # h2: land q at 0-63 / k at 64-127, then duplicate to
                        # the opposite half via SBUF->SBUF DMA.
                        qt = qt_tiles[tci]
                        nc.vector.tensor_copy(qt[0:64, 1, :], ps[0:64, :])
                        nc.vector.tensor_copy(kT_sb[64:128, 1, tcs], ps[64:128, :])
                        nc.sync.dma_start(qt[64:128, 1, :], qt[0:64, 1, :])
                        nc.sync.dma_start(kT_sb[0:64, 1, tcs], kT_sb[64:128, 1, tcs])

                for m in range(3):
                    fs.append(("qkvm", tci, lambda ps=None, m=m: gm(m, ps)))

                # v channels
                def gv(tb, psv=None, xt=xt, tci=tci):
                    if psv is None:
                        psv = qppool.tile([128, 512], F32, tag="qp", name="psv")
                    for cc in range(6):
                        nc.tensor.matmul(
                            psv[:, :192],
                            lhsT=xt[:, cc, tb * 128:(tb + 1) * 128],
                            rhs=wv_sb[:, cc, :],
                            start=(cc == 0), stop=(cc == 5),
                        )
                    for h in range(3):
                        nc.vector.tensor_copy(
                            v_sb[h][:, tci * 4 + tb, 0:64],
                            psv[:, 64 * h:64 * h + 64],
                        )

                for tb in range(4):
                    fs.append(("qkvv", tci, lambda ps=None, tb=tb: gv(tb, ps)))
                return fs

            # ---- output projection chains (fillers, chunk qc) ----
            def make_proj_fillers(qc):
                ob_tiles: dict[int, object] = {}

                def gp(tbl, n0, nsz, pp=None, qc=qc):
                    tbs = slice((4 * qc + tbl) * 128, (4 * qc + tbl + 1) * 128)
                    lbs = slice(tbl * 128, (tbl + 1) * 128)
                    ot = ot_tiles[qc]
                    if pp is None:
                        pp = qppool.tile([128, 512], F32, tag="qp", name="pp")
                    nc.tensor.matmul(
                        pp[:, :nsz],
                        lhsT=ot[:, 0, lbs],
                        rhs=wp0_sb[:, n0:n0 + nsz],
                        start=True, stop=False,
                    )
                    nc.tensor.matmul(
                        pp[:, :nsz],
                        lhsT=ot[:, 1, lbs],
                        rhs=wp1_sb[:, n0:n0 + nsz],
                        start=False, stop=True,
                    )
                    if n0 == 0:
                        ob_tiles[tbl] = osbpool.tile(
                            [128, 768], F32, tag="osb", name="ob"
                        )
                    ob = ob_tiles[tbl]
                    nc.vector.tensor_copy(ob[:, n0:n0 + nsz], pp[:, :nsz])
                    if n0 == 512:
                        nc.sync.dma_start(out[tbs, :], ob)

                fs = []
                for tbl in range(4):
                    fs.append(("proj", qc,
                               lambda ps=None, tbl=tbl: gp(tbl, 0, 512, ps)))
                    fs.append(("proj", qc,
                               lambda ps=None, tbl=tbl: gp(tbl, 512, 256, ps)))
                return fs

            # ---- S^T granule: all 3 heads' scores for one k-block ----
            # slot h of the [128, 3, 512] psum tile holds head h.  Emission
            # order alternates PE row groups (even granule: h0@0, h1@64,
            # h2@0; odd: h1@64, h0@0, h2@64) so adjacent matmuls overlap.
            gpar = [0]

            def emit_granule(qc, kb):
                qs = max(0, (kb - 4 * qc) * 128)
                stg = stpool.tile([128, 3, 512], F32, tag="stg", name="stg")
                kbs = slice(kb * 128, (kb + 1) * 128)
                if gpar[0] == 0:
                    order = ((0, 0, 0), (1, 0, 64), (2, 1, 0))
                else:
                    order = ((1, 0, 64), (0, 0, 0), (2, 1, 64))
                gpar[0] ^= 1
                for h, slab, base in order:
                    nc.tensor.matmul(
                        stg[:, h, qs:512],
                        lhsT=kT_sb[base:base + 64, slab, kbs],
                        rhs=qt_tiles[qc][base:base + 64, slab, qs:512],
                        start=True, stop=True,
                    )
                pt = ppool.tile([128, 3, 512], BF16, tag="pt", name="pt")
                nc.scalar.activation(
                    pt[:, :, qs:512], stg[:, :, qs:512], AF.Exp, scale=0.125
                )
                if kb >= 4 * qc:
                    # block straddling the causal diagonal: staircase mask
                    # (on the near-idle GpSimd so the DVE FIFO, which the
                    # projection chains wait on, stays short)
                    for h in range(3):
                        nc.gpsimd.tensor_mul(
                            pt[:, h, qs:qs + 128], pt[:, h, qs:qs + 128], emask
                        )
                pt_tiles[(qc, kb)] = (pt, qs)
                gidx[0] += 1

            # ---- PV accumulation chains + normalization ----
            open_pso: dict[tuple, object] = {}

            def pv_piece(h, cqc, kb0, kb1, acc=None):
                cnkb = 4 * (cqc + 1)
                if kb0 == 0:
                    open_pso[(h, cqc)] = (
                        acc if acc is not None else
                        psopool.tile([128, 512], F32, tag="pso", name="pso")
                    )
                pso = open_pso[(h, cqc)]
                for kb in range(kb0, kb1):
                    pt, qs = pt_tiles[(cqc, kb)]
                    nc.tensor.matmul(
                        pso[0:65, qs:512],
                        lhsT=v_sb[h][:, kb, :],
                        rhs=pt[:, h, qs:512],
                        start=(kb == 0), stop=(kb == cnkb - 1),
                        skip_group_check=True,
                    )
                if kb1 == cnkb:
                    ocp = spool.tile([128, 512], F32, tag="ocp")
                    nc.vector.tensor_copy(ocp[0:65, :], pso[0:65, :])
                    norm_head(h, cqc, ocp)
                    norms_done[(h, cqc)] = gidx[0]

            def norm_head(h, qc, ocp):
                # reciprocal of the 512-wide sums row: spread it over 64
                # partitions via SBUF DMA so the iterative DVE reciprocal
                # runs ~64x faster than on a single-partition row
                ot = ot_tiles[qc]
                lsplit = spool.tile([64, 8], F32, tag="lsplit")
                nc.sync.dma_start(lsplit, ocp[64:65, :])
                lrec = spool.tile([64, 8], F32, tag="lrec")
                nc.vector.reciprocal(lrec, lsplit)
                lrow = spool.tile([1, 512], F32, tag="lrow")
                nc.sync.dma_start(lrow, lrec)
                bc = spool.tile([64, 512], F32, tag="bc")
                nc.gpsimd.partition_broadcast(bc, lrow)
                if h == 1:
                    stg2 = spool.tile([64, 512], BF16, tag="stg2")
                    nc.vector.tensor_mul(stg2, ocp[0:64, :], bc)
                    nc.sync.dma_start(ot[64:128, 0, :], stg2)
                else:
                    slab = 0 if h == 0 else 1
                    nc.vector.tensor_mul(ot[0:64, slab, :], ocp[0:64, :], bc)

            # ---- global conveyor over chunks ----
            # Startup: chunk 0's q/k chains borrow the (still idle) granule
            # PSUM banks so the granule conveyor starts after only ~3 chain
            # latencies; everything else trickles in as fillers.
            f0 = make_qkv_fillers(0, xt_pre=xt0)
            fillers: list = f0[3:]       # v-chains of chunk 0
            if TCH > 1:
                fillers.extend(make_qkv_fillers(1))
            stg_a = stpool.tile([128, 3, 512], F32, tag="stg", name="stg")
            for i, (_, _, f) in enumerate(f0[:3]):
                f(stg_a[:, i, :])

            norms_done: dict = {}
            gidx = [0]                   # global granule counter

            def pop_filler():
                # a proj(c) chain reads ot(c), written by the h1/h2 chain
                # norms that are carried into chunk c+1 — only emit it once
                # those norms are emitted (tile deps are emission-ordered)
                # and the DVE has had a couple of granules to drain them.
                for i, (k, c, f) in enumerate(fillers):
                    if k != "proj" or ((1, c) in norms_done
                                       and (2, c) in norms_done):
                        fillers.pop(i)
                        f()
                        return

            def drain_qkv_upto(qc, kinds=("qkvm", "qkvv")):
                # correctness barrier: chunk qc's granules read qt(qc)/kT(qc)
                # (gm chains) and its PV reads v(qc) (gv chains); those must
                # be EMITTED before their consumers.
                while any(k in kinds and c <= qc for k, c, _ in fillers):
                    pop_filler()

            # PV chains of chunk qc-1 (h1, h2) carried into chunk qc's stream
            carry: list = []

            for qc in range(TCH):
                nkb = 4 * (qc + 1)
                if qc + 2 <= TCH - 1:
                    fillers.extend(make_qkv_fillers(qc + 2))
                if qc >= 1:
                    fillers.extend(make_proj_fillers(qc - 1))

                # attention output, transposed: slab0 = [h0|h1], slab1 = [h2|0]
                ot = otpool.tile([128, 2, 512], BF16, tag="ot", name="ot")
                ot_tiles[qc] = ot
                # wp1 rows 64-127 are zero so slab1's lower half is never
                # read with nonzero weight, but stale SBUF could hold NaN
                # patterns; keep it zeroed.
                nc.gpsimd.memset(ot[64:128, 1, :], 0.0)

                # PV weave: (ready_granule, head, chunk, piece).  Carried
                # chains are ready immediately; h0 pieces lag one granule
                # behind the granule producing their last pt.
                last = qc == TCH - 1

                def pieces_of(cnkb, w=8):
                    return [(w * j, min(w * j + w, cnkb))
                            for j in range((cnkb + w - 1) // w)]

                # weave queue: (ready_granule, h, chunk, kb0, kb1).  Carried
                # chains are ready immediately; same-chunk pieces lag two
                # granules behind the granule producing their last pt.
                pvq = [(0, h, cqc, a, b, False) for (h, cqc) in carry
                       for (a, b) in pieces_of(4 * (cqc + 1))]
                pvq += [(b + 2, 0, qc, a, b, False) for (a, b) in pieces_of(nkb)]
                if last:
                    # no next chunk to carry into: weave h2's chain too, on
                    # the (otherwise idle) chain bank; h1 + projections drain
                    # in the tail
                    pvq += [(b + 2, 2, qc, a, b, True)
                            for (a, b) in pieces_of(nkb)]
                    pvq.sort(key=lambda t: t[0])
                carry = []
                pi = 0
                drain_qkv_upto(qc, kinds=("qkvm",))
                if qc >= 1:
                    drain_qkv_upto(qc - 1, kinds=("qkvv",))

                for kb in range(0, nkb, 2):
                    # granule pairs: 6 adjacent S^T matmuls = 3 overlapped
                    # row-group pairs, amortizing the pipe drain over longer
                    # runs before PV/filler work interleaves
                    emit_granule(qc, kb)
                    emit_granule(qc, kb + 1)
                    if kb == 2:
                        # v(qc) must be emitted before h0(qc)'s first piece
                        drain_qkv_upto(qc, kinds=("qkvv",))
                    if not last and fillers and (
                            kb % 4 == 0 or len(fillers) > nkb - kb):
                        pop_filler()
                    n = 0
                    while pi < len(pvq) and n < 2 and pvq[pi][0] <= kb + 1:
                        r, h, cqc, a, b, onq = pvq[pi]
                        acc = (qppool.tile([128, 512], F32, tag="qp",
                                           name="pso")
                               if onq and a == 0 else None)
                        pv_piece(h, cqc, a, b, acc=acc)
                        pi += 1
                        n += 1
                # drain backlog (h0's and h2's final pieces land here)
                while pi < len(pvq):
                    r, h, cqc, a, b, onq = pvq[pi]
                    acc = (qppool.tile([128, 512], F32, tag="qp", name="pso")
                           if onq and a == 0 else None)
                    pv_piece(h, cqc, a, b, acc=acc)
                    pi += 1
                    if not last:
                        pop_filler()

                if not last:
                    carry = [(1, qc), (2, qc)]

            # ---- tail: last chunk's h1 chain + final projections ----
            # The granule conveyor is done, so its six PSUM banks are free:
            # h1 accumulates on one, and the proj(TCH-2)/proj(TCH-1) chains
            # round-robin over the rest with direct PSUM->DRAM evacuation
            # instead of serializing on the one chain bank.
            qcl = TCH - 1
            nkb = 4 * TCH
            tail_a = stpool.tile([128, 3, 512], F32, tag="stg", name="stg")
            tail_b = stpool.tile([128, 3, 512], F32, tag="stg", name="stg")
            pslots = [tail_a[:, 1, :], tail_a[:, 2, :]] + [
                tail_b[:, s, :] for s in range(3)]
            si = [0]

            def pop_filler_slot():
                for i, (k, c, f) in enumerate(fillers):
                    if k != "proj" or ((1, c) in norms_done
                                       and (2, c) in norms_done):
                        fillers.pop(i)
                        f(pslots[si[0] % len(pslots)])
                        si[0] += 1
                        return

            for a, b in pieces_of(nkb):
                pv_piece(1, qcl, a, b, acc=tail_a[:, 0, :])
                pop_filler_slot()
            while fillers:
                pop_filler_slot()
            for _, _, f in make_proj_fillers(qcl):
                f(pslots[si[0] % len(pslots)])
                si[0] += 1

    nc.compile()
    return nc


_NC_CACHE: dict[int, bass.Bass] = {}


def get_nc(seq_len: int) -> bass.Bass:
    if seq_len not in _NC_CACHE:
        _NC_CACHE[seq_len] = build_nc(seq_len)
    return _NC_CACHE[seq_len]


def make_in_maps(x: np.ndarray, w_attn: np.ndarray, w_proj: np.ndarray):
    """Per-core input dicts. Core c: batch c//4, head group c%4 (3 heads)."""
    bf16 = ml_dtypes.bfloat16
    in_maps = []
    for c in range(N_CORES):
        b, hg = divmod(c, 4)
        q = w_attn[192 * hg: 192 * hg + 192]
        k = w_attn[768 + 192 * hg: 768 + 192 * hg + 192]
        v = w_attn[1536 + 192 * hg: 1536 + 192 * hg + 192]
        wqk = np.concatenate([q[0:128], k[0:128], q[128:192], k[128:192]], axis=0)
        in_maps.append({
            "xT": np.ascontiguousarray(x[b].T).astype(bf16),
            "wqkT": np.ascontiguousarray(wqk.T).astype(bf16),
            "wvT": np.ascontiguousarray(v.T).astype(bf16),
            "wpT": np.ascontiguousarray(
                w_proj[:, 192 * hg: 192 * hg + 192].T
            ).astype(bf16),
        })
    return in_maps


def run_on_cores(x, w_attn, w_proj, trace: bool = False):
    from concourse.bass_utils import run_bass_kernel_spmd

    x = np.asarray(x, dtype=np.float32)
    w_attn = np.asarray(w_attn, dtype=np.float32)
    w_proj = np.asarray(w_proj, dtype=np.float32)
    nc = get_nc(x.shape[1])
    in_maps = make_in_maps(x, w_attn, w_proj)
    res = run_bass_kernel_spmd(
        nc, in_maps, core_ids=list(range(N_CORES)), trace=trace
    )
    outs = [r["out"] for r in res.results]
    full = np.stack(
        [sum(outs[4 * b + hg] for hg in range(4)) for b in range(B)], axis=0
    )
    return full, res


def kernel(x, w_attn, w_proj):
    full, _ = run_on_cores(x, w_attn, w_proj, trace=False)
    return full
